# revision 14
# baseline (speedup 1.0000x reference)
"""Bahdanau-style additive attention kernel for Trainium2 (8 NeuronCores).

reference:
    q_h    = relu(query @ w1.T)                      (B, H)
    k_h    = relu(key @ w2.T)                        (B, T, H)
    scores = tanh(q_h[:, None, :] + k_h) @ w_out     (B, T)
    attn   = softmax(scores, axis=1)
    out    = einsum('bt,bth->bh', attn, key)         (B, H)

B=128, T=2048, H=512. Pure data parallel: 16 batch rows per core.

Device strategy (per core):
  pass 1: the k_h matmul runs in fp8e4m3 DoubleRow mode (contraction over
  256 h per instruction at 0.5 cyc/row -> 4x bf16 FLOP rate).  The host
  pre-transposes key -> keyT [b, h, t] fp8 so the contraction dim h sits
  on partitions.  tanh(k_h_raw + q_h) is ONE ScalarE activation reading
  PSUM with per-partition bias q_h; the relu is folded in afterwards via
  the exact identity tanh(relu(k) + q) = max(tanh(k + q), tanh(q)) for
  q >= 0, which runs on the DVE in bf16 at 4x rate.  q_h and tanh(q_h)
  are precomputed on the host (tiny).  scores = w_out . th stays a bf16
  M=1 matmul accumulated over the four g-subtiles.
  pass 2: softmax runs partition-parallel on [GROUP, T] row groups; attn
  is transposed on-chip with PE transposes; out = attn @ key uses the
  natural-layout bf16 key tile as the STATIONARY operand (lhsT) and the
  attn column [128t, 1] as the moving operand, so each matmul streams
  only 1 row.  Each PSUM 2KB zero-region hosts only one accumulation
  group at a time (hardware constraint): every output column finishes
  its 16-step t accumulation before the next column starts.

  DMA: one whole-row keyT DMA (fp8, 2.9us) and one whole-row natural key
  DMA (bf16, 5.8us) per batch row, alternated between the SP and Pool
  queues (a DMA holds its issuing queue until the transfer completes, so
  two queues must interleave to keep the DMA engines saturated).  The
  natural-key tile for pass 2 is prefetched during pass 1.  Groups are
  software-pipelined (pass2 of group g is emitted after pass1 of group
  g+1) so the PE never waits on a softmax chain.
"""

import numpy as np
import ml_dtypes

import concourse.bass as bass
from concourse import bacc
import concourse.mybir as mybir
import concourse.tile as tile
from concourse import bass_utils

B, T, H = 128, 2048, 512
NCORES = 8
BPC = B // NCORES          # 16 batch rows per core
P = 128
HS = H // P                # 4 subtiles of the h/g dims
TC = 512                   # psum-bank sized t-chunk (fp32)
NCH = T // TC              # 4 chunks
GROUP = 4                  # batch rows per softmax group
NG = BPC // GROUP          # 4 groups
TP = T // P                # 16 t-subtiles for pass 2

f32 = mybir.dt.float32
bf16 = mybir.dt.bfloat16
f8 = mybir.dt.float8e4
AF = mybir.ActivationFunctionType
AX = mybir.AxisListType
PM = mybir.MatmulPerfMode

_CACHE = {}


def _build_nc():
    nc = bacc.Bacc(trn_type="TRN2", target_bir_lowering=False)

    keyT_f8 = nc.dram_tensor("keyT_f8", [BPC, H, T], f8, kind="ExternalInput")
    key_bf = nc.dram_tensor("key_bf", [BPC, T, H], bf16, kind="ExternalInput")
    w2T_f8 = nc.dram_tensor("w2T_f8", [H, H], f8, kind="ExternalInput")
    qhT = nc.dram_tensor("qhT", [H, BPC], f32, kind="ExternalInput")
    tqT = nc.dram_tensor("tqT", [H, BPC], f32, kind="ExternalInput")
    wout = nc.dram_tensor("wout", [H, 1], bf16, kind="ExternalInput")
    ident = nc.dram_tensor("ident", [P, P], bf16, kind="ExternalInput")
    out = nc.dram_tensor("out", [BPC, H], f32, kind="ExternalOutput")

    with tile.TileContext(nc) as tc:
        with (
            tc.tile_pool(name="const", bufs=1) as cpool,
            tc.tile_pool(name="kt", bufs=3) as kt_pool,
            tc.tile_pool(name="kb", bufs=6) as kb_pool,
            tc.tile_pool(name="u", bufs=3) as u_pool,
            tc.tile_pool(name="th", bufs=3) as th_pool,
            tc.tile_pool(name="stage", bufs=4) as stage_pool,
            tc.tile_pool(name="sm", bufs=2) as sm_pool,
            tc.tile_pool(name="osb", bufs=2) as out_pool,
            tc.tile_pool(name="ps_kh", bufs=2, space="PSUM") as ps_kh,
            tc.tile_pool(name="ps_sc", bufs=1, space="PSUM") as ps_sc,
            tc.tile_pool(name="ps_at", bufs=1, space="PSUM") as ps_at,
            tc.tile_pool(name="ps_o", bufs=1, space="PSUM") as ps_o,
        ):
            # ---- constants ----
            w2T_sb = cpool.tile([P, HS, H], f8)        # [p, h_sub, g]
            nc.sync.dma_start(w2T_sb[:], w2T_f8.ap().rearrange("(s p) g -> p s g", p=P))
            qhT_sb = cpool.tile([P, HS, BPC], f32)     # [g_sub, gs, b]
            nc.gpsimd.dma_start(qhT_sb[:], qhT.ap().rearrange("(s p) b -> p s b", p=P))
            tqT_sb = cpool.tile([P, HS, BPC], f32)     # tanh(q_h), same layout
            nc.gpsimd.dma_start(tqT_sb[:], tqT.ap().rearrange("(s p) b -> p s b", p=P))
            wout_sb = cpool.tile([P, HS], bf16)        # [p, g_sub]
            nc.gpsimd.dma_start(wout_sb[:], wout.ap().rearrange("(s p) o -> p (s o)", p=P))
            ident_sb = cpool.tile([P, P], bf16)
            nc.gpsimd.dma_start(ident_sb[:], ident.ap())

            kb_tiles = {}
            pending_stage = []   # (queue, scores_sb, j, stage) deferred DMAs
            pending_out = []     # (g, osb) deferred output DMAs

            def flush_stage(keep_last):
                # emit deferred stage->scores DMAs whose data is already
                # computed, so they never block their queue head.
                while len(pending_stage) > keep_last:
                    q, ssb, j, st = pending_stage.pop(0)
                    q.dma_start(ssb[j:j + 1, :], st[:])

            def flush_out(keep_last):
                while len(pending_out) > keep_last:
                    go, osb = pending_out.pop(0)
                    nc.sync.dma_start(
                        out.ap()[go * GROUP:(go + 1) * GROUP].rearrange(
                            "b (c p) -> p b c", p=P),
                        osb[:],
                    )

            def emit_b(b, scores_sb, j):
                qa = nc.sync      # kt + stage + out queue
                qb = nc.gpsimd    # kb queue
                # whole-row keyT load (fp8, 8KB/partition)
                kt = kt_pool.tile([P, HS, NCH, TC], f8, tag="kt")
                qa.dma_start(
                    kt[:],
                    keyT_f8.ap()[b].rearrange(
                        "(s p) (c t) -> p s c t", p=P, c=NCH),
                )
                # prefetch the natural-layout row for pass 2
                kb = kb_pool.tile([P, TP, H], bf16, tag="kb")
                qb.dma_start(
                    kb[:],
                    key_bf.ap()[b].rearrange("(c p) h -> p c h", p=P),
                )
                kb_tiles[b] = kb

                stage = stage_pool.tile([1, T], f32, tag="stage")
                for cc in range(NCH // 2):         # pairs of t-chunks
                    pss = []
                    for ci in range(2):
                        ps_sc_c = ps_sc.tile(
                            [1, TC], f32, tag="sc%d" % ci, name="psc%d" % ci)
                        pss.append(ps_sc_c)
                    for gs in range(HS):
                        ps = ps_kh.tile([P, 2, TC], f32, tag="kh")
                        for ci in range(2):
                            for i in range(2):     # hs pairs (DoubleRow)
                                nc.tensor.matmul(
                                    ps[:, ci, :],
                                    lhsT=w2T_sb[:, 2 * i:2 * i + 2,
                                                gs * P:(gs + 1) * P],
                                    rhs=kt[:, 2 * i:2 * i + 2, cc * 2 + ci, :],
                                    start=(i == 0),
                                    stop=(i == 1),
                                    perf_mode=PM.DoubleRow,
                                )
                        # tanh(k_raw + q); relu folded in via max with
                        # tanh(q) below (exact for q >= 0)
                        u = u_pool.tile([P, 2, TC], bf16, tag="u")
                        nc.scalar.activation(
                            u[:], ps[:], AF.Tanh,
                            bias=qhT_sb[:, gs, b:b + 1],
                        )
                        th = th_pool.tile([P, 2, TC], bf16, tag="th")
                        nc.vector.tensor_scalar_max(
                            th[:], u[:], tqT_sb[:, gs, b:b + 1])
                        for ci in range(2):
                            nc.tensor.matmul(
                                pss[ci][:],
                                lhsT=wout_sb[:, gs:gs + 1],
                                rhs=th[:, ci, :],
                                start=(gs == 0),
                                stop=(gs == HS - 1),
                            )
                    for ci in range(2):
                        c = cc * 2 + ci
                        nc.vector.tensor_copy(
                            stage[:, c * TC:(c + 1) * TC], pss[ci][:])
                pending_stage.append((qa, scores_sb, j, stage))

            def softmax_max(scores_sb):
                # partition-parallel softmax over T; no PE instructions here.
                mx = sm_pool.tile([GROUP, 1], f32, tag="mx")
                nc.vector.reduce_max(mx[:], scores_sb[:], axis=AX.X)
                nmx = sm_pool.tile([GROUP, 1], f32, tag="nmx")
                nc.vector.tensor_scalar_mul(nmx[:], mx[:], -1.0)
                return nmx

            def softmax_exp(scores_sb, nmx):
                # exp is written as unnormalized bf16 attn and normalized
                # in place on the DVE (bf16 sbuf -> 4x mode).
                attn = sm_pool.tile([GROUP, T], bf16, tag="attn")
                sums = sm_pool.tile([GROUP, 1], f32, tag="sums")
                nc.scalar.activation(
                    attn[:], scores_sb[:], AF.Exp, bias=nmx[:], accum_out=sums[:]
                )
                return attn, sums

            def softmax_norm(attn, sums):
                inv = sm_pool.tile([GROUP, 1], f32, tag="inv")
                nc.vector.reciprocal(inv[:], sums[:])
                nc.vector.tensor_scalar_mul(attn[:], attn[:], inv[:, 0:1])
                return attn

            def pass2(g, attn):
                # transpose attn on-chip: [GROUP, T] -> [128, TP, GROUP]
                pat = ps_at.tile([P, TP, GROUP], bf16, tag="at")
                for c2 in range(TP):
                    nc.tensor.transpose(
                        pat[:, c2, :],
                        attn[:, c2 * P:(c2 + 1) * P],
                        ident_sb[0:GROUP, 0:GROUP],
                    )
                attnT = sm_pool.tile([P, TP, GROUP], bf16, tag="attnT")
                nc.vector.tensor_copy(attnT[:], pat[:])

                # one accumulation group per PSUM 2KB zero-region at a time:
                # finish each output column over all 16 t-subtiles before
                # starting the next.
                osb = out_pool.tile([P, GROUP, HS], f32, tag="osb")
                for j in range(GROUP):
                    b = g * GROUP + j
                    kb = kb_tiles.pop(b)
                    pso = ps_o.tile([P, HS], f32, tag="o")
                    for hc in range(HS):
                        for c2 in range(TP):
                            nc.tensor.matmul(
                                pso[:, hc:hc + 1],
                                lhsT=kb[:, c2, hc * P:(hc + 1) * P],
                                rhs=attnT[:, c2, j:j + 1],
                                start=(c2 == 0),
                                stop=(c2 == TP - 1),
                            )
                    nc.vector.tensor_copy(osb[:, j, :], pso[:])
                pending_out.append((g, osb))

            # software-pipelined group loop: the softmax chain of group g-1
            # is staggered across the b-iterations of group g (max at j==1,
            # exp at j==2, normalize+pass2 at j==3) so neither the ACT nor
            # the PE stream ever head-blocks on it.
            scores_tiles = {}
            sm_state = {}
            for g in range(NG):
                scores_tiles[g] = sm_pool.tile(
                    [GROUP, T], f32, tag="scores", name="scores%d" % g)
                for j in range(GROUP):
                    b = g * GROUP + j
                    if g > 0:
                        if j == 1:
                            flush_stage(keep_last=1)
                            flush_out(keep_last=0)
                            sm_state["nmx"] = softmax_max(scores_tiles[g - 1])
                        elif j == 2:
                            sm_state["attn"], sm_state["sums"] = softmax_exp(
                                scores_tiles.pop(g - 1), sm_state.pop("nmx"))
                        elif j == 3:
                            attn = softmax_norm(
                                sm_state.pop("attn"), sm_state.pop("sums"))
                            pass2(g - 1, attn)
                    flush_stage(keep_last=2)
                    emit_b(b, scores_tiles[g], j)
            flush_stage(keep_last=0)
            nmx = softmax_max(scores_tiles[NG - 1])
            attn, sums = softmax_exp(scores_tiles.pop(NG - 1), nmx)
            attn = softmax_norm(attn, sums)
            pass2(NG - 1, attn)
            flush_out(keep_last=0)

    nc.compile()
    return nc


def kernel(query, key, w1, w2, w_out):
    query = np.asarray(query, dtype=np.float32)
    key = np.asarray(key, dtype=np.float32)
    w1 = np.asarray(w1, dtype=np.float32)
    w2 = np.asarray(w2, dtype=np.float32)
    w_out = np.asarray(w_out, dtype=np.float32)

    if "nc" not in _CACHE:
        _CACHE["nc"] = _build_nc()
    nc = _CACHE["nc"]

    f8np = ml_dtypes.float8_e4m3
    w2T_f8 = np.ascontiguousarray(w2.T).astype(f8np)
    wout_col = np.ascontiguousarray(w_out.reshape(H, 1)).astype(ml_dtypes.bfloat16)
    ident = np.eye(P, dtype=ml_dtypes.bfloat16)

    # host-side q_h = relu(query @ w1.T) and tanh(q_h), transposed [H, B]
    qh = np.maximum(query @ w1.T, 0.0).astype(np.float32)
    qhT = np.ascontiguousarray(qh.T)
    tqT = np.ascontiguousarray(np.tanh(qh.T)).astype(np.float32)

    in_maps = []
    for c in range(NCORES):
        sl = slice(c * BPC, (c + 1) * BPC)
        key_c = key[sl]
        in_maps.append({
            "keyT_f8": np.ascontiguousarray(
                key_c.transpose(0, 2, 1)).astype(f8np),
            "key_bf": np.ascontiguousarray(key_c).astype(ml_dtypes.bfloat16),
            "w2T_f8": w2T_f8,
            "qhT": np.ascontiguousarray(qhT[:, sl]),
            "tqT": np.ascontiguousarray(tqT[:, sl]),
            "wout": wout_col,
            "ident": ident,
        })

    _CACHE["in_maps"] = in_maps
    res = None
    last_exc = None
    for _attempt in range(3):
        try:
            res = bass_utils.run_bass_kernel_spmd(
                nc, in_maps, core_ids=list(range(NCORES)), trace=False)
            break
        except Exception as e:  # transient device wedge: retry
            last_exc = e
            import time as _time
            _time.sleep(2.0)
    if res is None:
        raise last_exc
    out = np.concatenate([r["out"] for r in res.results], axis=0)
    return out.astype(np.float32)


# revision 16
# speedup vs baseline: 1.0063x; 1.0063x over previous
"""Bahdanau-style additive attention kernel for Trainium2 (8 NeuronCores).

reference:
    q_h    = relu(query @ w1.T)                      (B, H)
    k_h    = relu(key @ w2.T)                        (B, T, H)
    scores = tanh(q_h[:, None, :] + k_h) @ w_out     (B, T)
    attn   = softmax(scores, axis=1)
    out    = einsum('bt,bth->bh', attn, key)         (B, H)

B=128, T=2048, H=512. Pure data parallel: 16 batch rows per core.

Device strategy (per core):
  pass 1: the k_h matmul runs in fp8e4m3 DoubleRow mode (contraction over
  256 h per instruction at 0.5 cyc/row -> 4x bf16 FLOP rate).  The host
  pre-transposes key -> keyT [b, h, t] fp8 so the contraction dim h sits
  on partitions.  tanh(k_h_raw + q_h) is ONE ScalarE activation reading
  PSUM with per-partition bias q_h; the relu is folded in afterwards via
  the exact identity tanh(relu(k) + q) = max(tanh(k + q), tanh(q)) for
  q >= 0, which runs on the DVE in bf16 at 4x rate.  q_h and tanh(q_h)
  are precomputed on the host (tiny).  scores = w_out . th stays a bf16
  M=1 matmul accumulated over the four g-subtiles.
  pass 2: softmax runs partition-parallel on [GROUP, T] row groups; attn
  is transposed on-chip with PE transposes; out = attn @ key uses the
  natural-layout bf16 key tile as the STATIONARY operand (lhsT) and the
  attn column [128t, 1] as the moving operand, so each matmul streams
  only 1 row.  Each PSUM 2KB zero-region hosts only one accumulation
  group at a time (hardware constraint): every output column finishes
  its 16-step t accumulation before the next column starts.

  DMA: one whole-row keyT DMA (fp8, 2.9us) and one whole-row natural key
  DMA (bf16, 5.8us) per batch row, alternated between the SP and Pool
  queues (a DMA holds its issuing queue until the transfer completes, so
  two queues must interleave to keep the DMA engines saturated).  The
  natural-key tile for pass 2 is prefetched during pass 1.  Groups are
  software-pipelined (pass2 of group g is emitted after pass1 of group
  g+1) so the PE never waits on a softmax chain.
"""

import numpy as np
import ml_dtypes

import concourse.bass as bass
from concourse import bacc
import concourse.mybir as mybir
import concourse.tile as tile
from concourse import bass_utils

B, T, H = 128, 2048, 512
NCORES = 8
BPC = B // NCORES          # 16 batch rows per core
P = 128
HS = H // P                # 4 subtiles of the h/g dims
TC = 512                   # psum-bank sized t-chunk (fp32)
NCH = T // TC              # 4 chunks
GROUP = 4                  # batch rows per softmax group
NG = BPC // GROUP          # 4 groups
TP = T // P                # 16 t-subtiles for pass 2

f32 = mybir.dt.float32
bf16 = mybir.dt.bfloat16
f8 = mybir.dt.float8e4
AF = mybir.ActivationFunctionType
AX = mybir.AxisListType
PM = mybir.MatmulPerfMode

_CACHE = {}


def _build_nc():
    nc = bacc.Bacc(trn_type="TRN2", target_bir_lowering=False)

    keyT_f8 = nc.dram_tensor("keyT_f8", [BPC, H, T], f8, kind="ExternalInput")
    key_bf = nc.dram_tensor("key_bf", [BPC, T, H], bf16, kind="ExternalInput")
    w2T_f8 = nc.dram_tensor("w2T_f8", [H, H], f8, kind="ExternalInput")
    qhT = nc.dram_tensor("qhT", [H, BPC], f32, kind="ExternalInput")
    tqT = nc.dram_tensor("tqT", [H, BPC], f32, kind="ExternalInput")
    wout = nc.dram_tensor("wout", [H, 1], bf16, kind="ExternalInput")
    ident = nc.dram_tensor("ident", [P, P], bf16, kind="ExternalInput")
    out = nc.dram_tensor("out", [BPC, H], f32, kind="ExternalOutput")

    with tile.TileContext(nc) as tc:
        with (
            tc.tile_pool(name="const", bufs=1) as cpool,
            tc.tile_pool(name="kt", bufs=4) as kt_pool,
            tc.tile_pool(name="kb", bufs=6) as kb_pool,
            tc.tile_pool(name="th", bufs=3) as th_pool,
            tc.tile_pool(name="stage", bufs=3) as stage_pool,
            tc.tile_pool(name="sm", bufs=2) as sm_pool,
            tc.tile_pool(name="osb", bufs=2) as out_pool,
            tc.tile_pool(name="ps_kh", bufs=2, space="PSUM") as ps_kh,
            tc.tile_pool(name="ps_sc", bufs=1, space="PSUM") as ps_sc,
            tc.tile_pool(name="ps_at", bufs=1, space="PSUM") as ps_at,
            tc.tile_pool(name="ps_o", bufs=1, space="PSUM") as ps_o,
        ):
            # ---- constants ----
            w2T_sb = cpool.tile([P, HS, H], f8)        # [p, h_sub, g]
            nc.sync.dma_start(w2T_sb[:], w2T_f8.ap().rearrange("(s p) g -> p s g", p=P))
            qhT_sb = cpool.tile([P, HS, BPC], f32)     # [g_sub, gs, b]
            nc.gpsimd.dma_start(qhT_sb[:], qhT.ap().rearrange("(s p) b -> p s b", p=P))
            tqT_sb = cpool.tile([P, HS, BPC], f32)     # tanh(q_h), same layout
            nc.gpsimd.dma_start(tqT_sb[:], tqT.ap().rearrange("(s p) b -> p s b", p=P))
            wout_sb = cpool.tile([P, HS], bf16)        # [p, g_sub]
            nc.gpsimd.dma_start(wout_sb[:], wout.ap().rearrange("(s p) o -> p (s o)", p=P))
            ident_sb = cpool.tile([P, P], bf16)
            nc.gpsimd.dma_start(ident_sb[:], ident.ap())

            kb_tiles = {}
            pending_stage = []   # (queue, scores_sb, j, stage) deferred DMAs
            pending_out = []     # (g, osb) deferred output DMAs

            def flush_stage(keep_last):
                # emit deferred stage->scores DMAs whose data is already
                # computed, so they never block their queue head.
                while len(pending_stage) > keep_last:
                    q, ssb, j, st = pending_stage.pop(0)
                    nc.scalar.dma_start(ssb[j:j + 1, :], st[:])

            def flush_out(keep_last):
                while len(pending_out) > keep_last:
                    go, osb = pending_out.pop(0)
                    nc.scalar.dma_start(
                        out.ap()[go * GROUP:(go + 1) * GROUP].rearrange(
                            "b (c p) -> p b c", p=P),
                        osb[:],
                    )

            def emit_b(b, scores_sb, j):
                qa = nc.sync      # kt + stage + out queue
                qb = nc.gpsimd    # kb queue
                # whole-row keyT load (fp8, 8KB/partition)
                kt = kt_pool.tile([P, HS, NCH, TC], f8, tag="kt")
                qa.dma_start(
                    kt[:],
                    keyT_f8.ap()[b].rearrange(
                        "(s p) (c t) -> p s c t", p=P, c=NCH),
                )
                # prefetch the natural-layout row for pass 2
                kb = kb_pool.tile([P, TP, H], bf16, tag="kb")
                qb.dma_start(
                    kb[:],
                    key_bf.ap()[b].rearrange("(c p) h -> p c h", p=P),
                )
                kb_tiles[b] = kb

                stage = stage_pool.tile([1, T], f32, tag="stage")
                for cc in range(NCH // 2):         # pairs of t-chunks
                    pss = []
                    for ci in range(2):
                        ps_sc_c = ps_sc.tile(
                            [1, TC], f32, tag="sc%d" % ci, name="psc%d" % ci)
                        pss.append(ps_sc_c)
                    for gs in range(HS):
                        ps = ps_kh.tile([P, 2, TC], f32, tag="kh")
                        for ci in range(2):
                            for i in range(2):     # hs pairs (DoubleRow)
                                nc.tensor.matmul(
                                    ps[:, ci, :],
                                    lhsT=w2T_sb[:, 2 * i:2 * i + 2,
                                                gs * P:(gs + 1) * P],
                                    rhs=kt[:, 2 * i:2 * i + 2, cc * 2 + ci, :],
                                    start=(i == 0),
                                    stop=(i == 1),
                                    perf_mode=PM.DoubleRow,
                                )
                        # tanh(k_raw + q); relu folded in via an in-place
                        # max with tanh(q) (exact for q >= 0)
                        th = th_pool.tile([P, 2, TC], bf16, tag="th")
                        nc.scalar.activation(
                            th[:], ps[:], AF.Tanh,
                            bias=qhT_sb[:, gs, b:b + 1],
                        )
                        nc.vector.tensor_scalar_max(
                            th[:], th[:], tqT_sb[:, gs, b:b + 1])
                        for ci in range(2):
                            nc.tensor.matmul(
                                pss[ci][:],
                                lhsT=wout_sb[:, gs:gs + 1],
                                rhs=th[:, ci, :],
                                start=(gs == 0),
                                stop=(gs == HS - 1),
                            )
                    for ci in range(2):
                        c = cc * 2 + ci
                        nc.vector.tensor_copy(
                            stage[:, c * TC:(c + 1) * TC], pss[ci][:])
                pending_stage.append((qa, scores_sb, j, stage))

            def softmax_max(scores_sb):
                # partition-parallel softmax over T; no PE instructions here.
                mx = sm_pool.tile([GROUP, 1], f32, tag="mx")
                nc.vector.reduce_max(mx[:], scores_sb[:], axis=AX.X)
                nmx = sm_pool.tile([GROUP, 1], f32, tag="nmx")
                nc.vector.tensor_scalar_mul(nmx[:], mx[:], -1.0)
                return nmx

            def softmax_exp(scores_sb, nmx):
                # exp is written as unnormalized bf16 attn and normalized
                # in place on the DVE (bf16 sbuf -> 4x mode).
                attn = sm_pool.tile([GROUP, T], bf16, tag="attn")
                sums = sm_pool.tile([GROUP, 1], f32, tag="sums")
                nc.scalar.activation(
                    attn[:], scores_sb[:], AF.Exp, bias=nmx[:], accum_out=sums[:]
                )
                return attn, sums

            def softmax_norm(attn, sums):
                inv = sm_pool.tile([GROUP, 1], f32, tag="inv")
                nc.vector.reciprocal(inv[:], sums[:])
                nc.vector.tensor_scalar_mul(attn[:], attn[:], inv[:, 0:1])
                return attn

            def pass2(g, attn):
                # transpose attn on-chip: [GROUP, T] -> [128, TP, GROUP]
                pat = ps_at.tile([P, TP, GROUP], bf16, tag="at")
                for c2 in range(TP):
                    nc.tensor.transpose(
                        pat[:, c2, :],
                        attn[:, c2 * P:(c2 + 1) * P],
                        ident_sb[0:GROUP, 0:GROUP],
                    )
                attnT = sm_pool.tile([P, TP, GROUP], bf16, tag="attnT")
                nc.vector.tensor_copy(attnT[:], pat[:])

                # one accumulation group per PSUM 2KB zero-region at a time:
                # finish each output column over all 16 t-subtiles before
                # starting the next.
                osb = out_pool.tile([P, GROUP, HS], f32, tag="osb")
                for j in range(GROUP):
                    b = g * GROUP + j
                    kb = kb_tiles.pop(b)
                    pso = ps_o.tile([P, HS], f32, tag="o")
                    for hc in range(HS):
                        for c2 in range(TP):
                            nc.tensor.matmul(
                                pso[:, hc:hc + 1],
                                lhsT=kb[:, c2, hc * P:(hc + 1) * P],
                                rhs=attnT[:, c2, j:j + 1],
                                start=(c2 == 0),
                                stop=(c2 == TP - 1),
                            )
                    nc.vector.tensor_copy(osb[:, j, :], pso[:])
                pending_out.append((g, osb))

            # software-pipelined group loop: the softmax chain of group g-1
            # is staggered across the b-iterations of group g (max at j==1,
            # exp at j==2, normalize+pass2 at j==3) so neither the ACT nor
            # the PE stream ever head-blocks on it.
            scores_tiles = {}
            sm_state = {}
            for g in range(NG):
                scores_tiles[g] = sm_pool.tile(
                    [GROUP, T], f32, tag="scores", name="scores%d" % g)
                for j in range(GROUP):
                    b = g * GROUP + j
                    if g > 0:
                        if j == 1:
                            flush_stage(keep_last=1)
                            flush_out(keep_last=0)
                            nmx = softmax_max(scores_tiles[g - 1])
                            sm_state["attn"], sm_state["sums"] = softmax_exp(
                                scores_tiles.pop(g - 1), nmx)
                        elif j == 2:
                            attn = softmax_norm(
                                sm_state.pop("attn"), sm_state.pop("sums"))
                            pass2(g - 1, attn)
                    flush_stage(keep_last=2)
                    emit_b(b, scores_tiles[g], j)
            flush_stage(keep_last=0)
            nmx = softmax_max(scores_tiles[NG - 1])
            attn, sums = softmax_exp(scores_tiles.pop(NG - 1), nmx)
            attn = softmax_norm(attn, sums)
            pass2(NG - 1, attn)
            flush_out(keep_last=0)

    nc.compile()
    return nc


def kernel(query, key, w1, w2, w_out):
    query = np.asarray(query, dtype=np.float32)
    key = np.asarray(key, dtype=np.float32)
    w1 = np.asarray(w1, dtype=np.float32)
    w2 = np.asarray(w2, dtype=np.float32)
    w_out = np.asarray(w_out, dtype=np.float32)

    if "nc" not in _CACHE:
        _CACHE["nc"] = _build_nc()
    nc = _CACHE["nc"]

    f8np = ml_dtypes.float8_e4m3
    w2T_f8 = np.ascontiguousarray(w2.T).astype(f8np)
    wout_col = np.ascontiguousarray(w_out.reshape(H, 1)).astype(ml_dtypes.bfloat16)
    ident = np.eye(P, dtype=ml_dtypes.bfloat16)

    # host-side q_h = relu(query @ w1.T) and tanh(q_h), transposed [H, B]
    qh = np.maximum(query @ w1.T, 0.0).astype(np.float32)
    qhT = np.ascontiguousarray(qh.T)
    tqT = np.ascontiguousarray(np.tanh(qh.T)).astype(np.float32)

    in_maps = []
    for c in range(NCORES):
        sl = slice(c * BPC, (c + 1) * BPC)
        key_c = key[sl]
        in_maps.append({
            "keyT_f8": np.ascontiguousarray(
                key_c.transpose(0, 2, 1)).astype(f8np),
            "key_bf": np.ascontiguousarray(key_c).astype(ml_dtypes.bfloat16),
            "w2T_f8": w2T_f8,
            "qhT": np.ascontiguousarray(qhT[:, sl]),
            "tqT": np.ascontiguousarray(tqT[:, sl]),
            "wout": wout_col,
            "ident": ident,
        })

    _CACHE["in_maps"] = in_maps
    res = None
    last_exc = None
    for _attempt in range(3):
        try:
            res = bass_utils.run_bass_kernel_spmd(
                nc, in_maps, core_ids=list(range(NCORES)), trace=False)
            break
        except Exception as e:  # transient device wedge: retry
            last_exc = e
            import time as _time
            _time.sleep(2.0)
    if res is None:
        raise last_exc
    out = np.concatenate([r["out"] for r in res.results], axis=0)
    return out.astype(np.float32)


# revision 17
# speedup vs baseline: 1.0969x; 1.0900x over previous
"""Bahdanau-style additive attention kernel for Trainium2 (8 NeuronCores).

reference:
    q_h    = relu(query @ w1.T)                      (B, H)
    k_h    = relu(key @ w2.T)                        (B, T, H)
    scores = tanh(q_h[:, None, :] + k_h) @ w_out     (B, T)
    attn   = softmax(scores, axis=1)
    out    = einsum('bt,bth->bh', attn, key)         (B, H)

B=128, T=2048, H=512. Pure data parallel: 16 batch rows per core.

Device strategy (per core):
  pass 1: the k_h matmul runs in fp8e4m3 DoubleRow mode (contraction over
  256 h per instruction at 0.5 cyc/row -> 4x bf16 FLOP rate).  The host
  pre-transposes key -> keyT [b, h, t] fp8 so the contraction dim h sits
  on partitions.  tanh(k_h_raw + q_h) is ONE ScalarE activation reading
  PSUM with per-partition bias q_h; the relu is folded in afterwards via
  the exact identity tanh(relu(k) + q) = max(tanh(k + q), tanh(q)) for
  q >= 0, which runs on the DVE in bf16 at 4x rate.  q_h and tanh(q_h)
  are precomputed on the host (tiny).  scores = w_out . th stays a bf16
  M=1 matmul accumulated over the four g-subtiles.
  pass 2: softmax runs partition-parallel on [GROUP, T] row groups; attn
  is transposed on-chip with PE transposes; out = attn @ key uses the
  natural-layout bf16 key tile as the STATIONARY operand (lhsT) and the
  attn column [128t, 1] as the moving operand, so each matmul streams
  only 1 row.  Each PSUM 2KB zero-region hosts only one accumulation
  group at a time (hardware constraint): every output column finishes
  its 16-step t accumulation before the next column starts.

  DMA: one whole-row keyT DMA (fp8, 2.9us) and one whole-row natural key
  DMA (bf16, 5.8us) per batch row, alternated between the SP and Pool
  queues (a DMA holds its issuing queue until the transfer completes, so
  two queues must interleave to keep the DMA engines saturated).  The
  natural-key tile for pass 2 is prefetched during pass 1.  Groups are
  software-pipelined (pass2 of group g is emitted after pass1 of group
  g+1) so the PE never waits on a softmax chain.
"""

import numpy as np
import ml_dtypes

import concourse.bass as bass
from concourse import bacc
import concourse.mybir as mybir
import concourse.tile as tile
from concourse import bass_utils

B, T, H = 128, 2048, 512
NCORES = 8
BPC = B // NCORES          # 16 batch rows per core
P = 128
HS = H // P                # 4 subtiles of the h/g dims
TC = 512                   # psum-bank sized t-chunk (fp32)
NCH = T // TC              # 4 chunks
GROUP = 4                  # batch rows per softmax group
NG = BPC // GROUP          # 4 groups
TP = T // P                # 16 t-subtiles for pass 2

f32 = mybir.dt.float32
bf16 = mybir.dt.bfloat16
f8 = mybir.dt.float8e4
AF = mybir.ActivationFunctionType
AX = mybir.AxisListType
PM = mybir.MatmulPerfMode

_CACHE = {}


def _build_nc():
    nc = bacc.Bacc(trn_type="TRN2", target_bir_lowering=False)

    keyT_f8 = nc.dram_tensor("keyT_f8", [BPC, H, T], f8, kind="ExternalInput")
    key_bf = nc.dram_tensor("key_bf", [BPC, T, H], bf16, kind="ExternalInput")
    w2T_f8 = nc.dram_tensor("w2T_f8", [H, H], f8, kind="ExternalInput")
    qhT = nc.dram_tensor("qhT", [H, BPC], f32, kind="ExternalInput")
    tqT = nc.dram_tensor("tqT", [H, BPC], f32, kind="ExternalInput")
    wout = nc.dram_tensor("wout", [H, 1], bf16, kind="ExternalInput")
    ident = nc.dram_tensor("ident", [P, P], bf16, kind="ExternalInput")
    out = nc.dram_tensor("out", [BPC, H], f32, kind="ExternalOutput")

    with tile.TileContext(nc) as tc:
        with (
            tc.tile_pool(name="const", bufs=1) as cpool,
            tc.tile_pool(name="kt", bufs=4) as kt_pool,
            tc.tile_pool(name="kb", bufs=6) as kb_pool,
            tc.tile_pool(name="th", bufs=3) as th_pool,
            tc.tile_pool(name="stage", bufs=3) as stage_pool,
            tc.tile_pool(name="sm", bufs=2) as sm_pool,
            tc.tile_pool(name="osb", bufs=2) as out_pool,
            tc.tile_pool(name="ps_kh", bufs=2, space="PSUM") as ps_kh,
            tc.tile_pool(name="ps_sc", bufs=1, space="PSUM") as ps_sc,
            tc.tile_pool(name="ps_at", bufs=1, space="PSUM") as ps_at,
            tc.tile_pool(name="ps_o", bufs=1, space="PSUM") as ps_o,
        ):
            # ---- constants ----
            w2T_sb = cpool.tile([P, HS, H], f8)        # [p, h_sub, g]
            nc.sync.dma_start(w2T_sb[:], w2T_f8.ap().rearrange("(s p) g -> p s g", p=P))
            qhT_sb = cpool.tile([P, HS, BPC], f32)     # [g_sub, gs, b]
            nc.gpsimd.dma_start(qhT_sb[:], qhT.ap().rearrange("(s p) b -> p s b", p=P))
            tqT_sb = cpool.tile([P, HS, BPC], f32)     # tanh(q_h), same layout
            nc.gpsimd.dma_start(tqT_sb[:], tqT.ap().rearrange("(s p) b -> p s b", p=P))
            wout_sb = cpool.tile([P, HS], bf16)        # [p, g_sub]
            nc.gpsimd.dma_start(wout_sb[:], wout.ap().rearrange("(s p) o -> p (s o)", p=P))
            ident_sb = cpool.tile([P, P], bf16)
            nc.gpsimd.dma_start(ident_sb[:], ident.ap())

            kb_tiles = {}
            pending_stage = []   # (queue, scores_sb, j, stage) deferred DMAs
            pending_out = []     # (g, osb) deferred output DMAs

            def flush_stage(keep_last):
                # emit deferred stage->scores DMAs whose data is already
                # computed, so they never block their queue head.
                while len(pending_stage) > keep_last:
                    q, ssb, j, st = pending_stage.pop(0)
                    nc.sync.dma_start(ssb[j:j + 1, :], st[:])

            def flush_out(keep_last):
                while len(pending_out) > keep_last:
                    go, osb = pending_out.pop(0)
                    nc.sync.dma_start(
                        out.ap()[go * GROUP:(go + 1) * GROUP].rearrange(
                            "b (c p) -> p b c", p=P),
                        osb[:],
                    )

            def emit_b(b, scores_sb, j):
                qa = nc.sync      # kt + stage + out queue
                qb = nc.gpsimd    # kb queue
                # whole-row keyT load (fp8, 8KB/partition)
                kt = kt_pool.tile([P, HS, NCH, TC], f8, tag="kt")
                qa.dma_start(
                    kt[:],
                    keyT_f8.ap()[b].rearrange(
                        "(s p) (c t) -> p s c t", p=P, c=NCH),
                )
                # prefetch the natural-layout row for pass 2
                kb = kb_pool.tile([P, TP, H], bf16, tag="kb")
                qb.dma_start(
                    kb[:],
                    key_bf.ap()[b].rearrange("(c p) h -> p c h", p=P),
                )
                kb_tiles[b] = kb

                stage = stage_pool.tile([1, T], f32, tag="stage")
                for cc in range(NCH // 2):         # pairs of t-chunks
                    pss = []
                    for ci in range(2):
                        ps_sc_c = ps_sc.tile(
                            [1, TC], f32, tag="sc%d" % ci, name="psc%d" % ci)
                        pss.append(ps_sc_c)
                    for gs in range(HS):
                        ps = ps_kh.tile([P, 2, TC], f32, tag="kh")
                        for ci in range(2):
                            for i in range(2):     # hs pairs (DoubleRow)
                                nc.tensor.matmul(
                                    ps[:, ci, :],
                                    lhsT=w2T_sb[:, 2 * i:2 * i + 2,
                                                gs * P:(gs + 1) * P],
                                    rhs=kt[:, 2 * i:2 * i + 2, cc * 2 + ci, :],
                                    start=(i == 0),
                                    stop=(i == 1),
                                    perf_mode=PM.DoubleRow,
                                )
                        # tanh(k_raw + q); relu folded in via an in-place
                        # max with tanh(q) (exact for q >= 0)
                        th = th_pool.tile([P, 2, TC], bf16, tag="th")
                        nc.scalar.activation(
                            th[:], ps[:], AF.Tanh,
                            bias=qhT_sb[:, gs, b:b + 1],
                        )
                        nc.vector.tensor_scalar_max(
                            th[:], th[:], tqT_sb[:, gs, b:b + 1])
                        for ci in range(2):
                            nc.tensor.matmul(
                                pss[ci][:],
                                lhsT=wout_sb[:, gs:gs + 1],
                                rhs=th[:, ci, :],
                                start=(gs == 0),
                                stop=(gs == HS - 1),
                            )
                    for ci in range(2):
                        c = cc * 2 + ci
                        nc.vector.tensor_copy(
                            stage[:, c * TC:(c + 1) * TC], pss[ci][:])
                pending_stage.append((qa, scores_sb, j, stage))

            def softmax_exp(scores_sb):
                # softmax without max-subtraction: scores = w_out . tanh(..)
                # are bounded by ||w_out||_1 (~4 in practice), so exp cannot
                # overflow fp32/bf16.  exp is written as unnormalized bf16
                # attn and normalized in place on the DVE (4x mode).
                attn = sm_pool.tile([GROUP, T], bf16, tag="attn")
                sums = sm_pool.tile([GROUP, 1], f32, tag="sums")
                nc.scalar.activation(
                    attn[:], scores_sb[:], AF.Exp, accum_out=sums[:]
                )
                return attn, sums

            def softmax_norm(attn, sums):
                inv = sm_pool.tile([GROUP, 1], f32, tag="inv")
                nc.vector.reciprocal(inv[:], sums[:])
                nc.vector.tensor_scalar_mul(attn[:], attn[:], inv[:, 0:1])
                return attn

            def pass2(g, attn):
                # transpose attn on-chip: [GROUP, T] -> [128, TP, GROUP]
                pat = ps_at.tile([P, TP, GROUP], bf16, tag="at")
                for c2 in range(TP):
                    nc.tensor.transpose(
                        pat[:, c2, :],
                        attn[:, c2 * P:(c2 + 1) * P],
                        ident_sb[0:GROUP, 0:GROUP],
                    )
                attnT = sm_pool.tile([P, TP, GROUP], bf16, tag="attnT")
                nc.vector.tensor_copy(attnT[:], pat[:])

                # one accumulation group per PSUM 2KB zero-region at a time:
                # finish each output column over all 16 t-subtiles before
                # starting the next.
                osb = out_pool.tile([P, GROUP, HS], f32, tag="osb")
                for j in range(GROUP):
                    b = g * GROUP + j
                    kb = kb_tiles.pop(b)
                    pso = ps_o.tile([P, HS], f32, tag="o")
                    for hc in range(HS):
                        for c2 in range(TP):
                            nc.tensor.matmul(
                                pso[:, hc:hc + 1],
                                lhsT=kb[:, c2, hc * P:(hc + 1) * P],
                                rhs=attnT[:, c2, j:j + 1],
                                start=(c2 == 0),
                                stop=(c2 == TP - 1),
                            )
                    nc.vector.tensor_copy(osb[:, j, :], pso[:])
                pending_out.append((g, osb))

            # software-pipelined group loop: the softmax chain of group g-1
            # is staggered across the b-iterations of group g (max at j==1,
            # exp at j==2, normalize+pass2 at j==3) so neither the ACT nor
            # the PE stream ever head-blocks on it.
            scores_tiles = {}
            sm_state = {}
            for g in range(NG):
                scores_tiles[g] = sm_pool.tile(
                    [GROUP, T], f32, tag="scores", name="scores%d" % g)
                for j in range(GROUP):
                    b = g * GROUP + j
                    if g > 0:
                        if j == 1:
                            flush_stage(keep_last=1)
                            flush_out(keep_last=0)
                            sm_state["attn"], sm_state["sums"] = softmax_exp(
                                scores_tiles.pop(g - 1))
                        elif j == 2:
                            attn = softmax_norm(
                                sm_state.pop("attn"), sm_state.pop("sums"))
                            pass2(g - 1, attn)
                    flush_stage(keep_last=1)
                    emit_b(b, scores_tiles[g], j)
            flush_stage(keep_last=0)
            attn, sums = softmax_exp(scores_tiles.pop(NG - 1))
            attn = softmax_norm(attn, sums)
            pass2(NG - 1, attn)
            flush_out(keep_last=0)

    nc.compile()
    return nc


def kernel(query, key, w1, w2, w_out):
    query = np.asarray(query, dtype=np.float32)
    key = np.asarray(key, dtype=np.float32)
    w1 = np.asarray(w1, dtype=np.float32)
    w2 = np.asarray(w2, dtype=np.float32)
    w_out = np.asarray(w_out, dtype=np.float32)

    if "nc" not in _CACHE:
        _CACHE["nc"] = _build_nc()
    nc = _CACHE["nc"]

    f8np = ml_dtypes.float8_e4m3
    w2T_f8 = np.ascontiguousarray(w2.T).astype(f8np)
    wout_col = np.ascontiguousarray(w_out.reshape(H, 1)).astype(ml_dtypes.bfloat16)
    ident = np.eye(P, dtype=ml_dtypes.bfloat16)

    # host-side q_h = relu(query @ w1.T) and tanh(q_h), transposed [H, B]
    qh = np.maximum(query @ w1.T, 0.0).astype(np.float32)
    qhT = np.ascontiguousarray(qh.T)
    tqT = np.ascontiguousarray(np.tanh(qh.T)).astype(np.float32)

    in_maps = []
    for c in range(NCORES):
        sl = slice(c * BPC, (c + 1) * BPC)
        key_c = key[sl]
        in_maps.append({
            "keyT_f8": np.ascontiguousarray(
                key_c.transpose(0, 2, 1)).astype(f8np),
            "key_bf": np.ascontiguousarray(key_c).astype(ml_dtypes.bfloat16),
            "w2T_f8": w2T_f8,
            "qhT": np.ascontiguousarray(qhT[:, sl]),
            "tqT": np.ascontiguousarray(tqT[:, sl]),
            "wout": wout_col,
            "ident": ident,
        })

    _CACHE["in_maps"] = in_maps
    res = None
    last_exc = None
    for _attempt in range(3):
        try:
            res = bass_utils.run_bass_kernel_spmd(
                nc, in_maps, core_ids=list(range(NCORES)), trace=False)
            break
        except Exception as e:  # transient device wedge: retry
            last_exc = e
            import time as _time
            _time.sleep(2.0)
    if res is None:
        raise last_exc
    out = np.concatenate([r["out"] for r in res.results], axis=0)
    return out.astype(np.float32)
